# revision 11
# baseline (speedup 1.0000x reference)
"""Trainium2 Bass kernel for nn_NewSplitRTrainer (streaming top-1 cosine search).

Math: the reference's streaming argmax + gather + differentiable re-projection
collapses (forward value) to
    loss = -(SD/HD) * sum_{t,u} mean_b max_{l in all keys} cos(q[t,u,b], k[t,u,l])
because the re-projected matched key in unit (t,u) is exactly the projection
whose cosine against q was maximized during the search (clips never bind for
randn inputs).  So the kernel computes per-(trial,unit,query) max cosine.

Sharding: the key/buffer axis (STEPS=8 blocks) across the 8 cores; each core
processes one 4096-key block for all trials/units, returns [16, 1024] partial
maxes; host max-reduces across cores and finishes the (tiny) scalar.
"""

import sys

for _p in ("/opt/trn_rl_repo", "/root/.axon_site/_ro/trn_rl_repo"):
    if _p not in sys.path:
        sys.path.append(_p)

import numpy as np
import ml_dtypes

import concourse.bass as bass  # noqa: F401  (registers AP machinery)
import concourse.mybir as mybir
from concourse import bacc
from concourse.tile import TileContext
from concourse.masks import make_identity
from concourse.bass_utils import run_bass_kernel_spmd

F32 = mybir.dt.float32
BF16 = mybir.dt.bfloat16
AF = mybir.ActivationFunctionType
BF = ml_dtypes.bfloat16

T, C, S = 4, 2, 2
U = C * S
HD, PD, SD = 1024, 512, 256
BZ, L, STEPS = 1024, 4096, 8
NCORES = 8

KH = HD // 128   # contraction chunks for previous_R matmuls
MC = HD // 128   # output-dim chunks of the rotated space
KP = PD // 128   # contraction chunks per prev-chunk rotation
QC = BZ // 128   # query chunks
KG = 8           # key groups per core
GK = L // KG     # keys per group
KC = GK // 128   # key-128-chunks per group


def build_program(n_cores=NCORES, n_kg=KG, use_ttr=False):
    nc = bacc.Bacc("TRN2", target_bir_lowering=False, debug=False,
                   num_devices=n_cores)
    kbT = nc.dram_tensor("kbT", [HD, L], BF16, kind="ExternalInput")
    R = nc.dram_tensor("R", [HD, HD], BF16, kind="ExternalInput")
    Rs = nc.dram_tensor("Rs", [T, C, PD, PD], BF16, kind="ExternalInput")
    hT = nc.dram_tensor("hT", [HD, BZ], BF16, kind="ExternalInput")
    # [query%128, (t,u,qchunk)] layout — contiguous per partition; host
    # reassembles to [T*U, BZ].
    y = nc.dram_tensor("y", [128, T * U * QC], F32, kind="ExternalOutput")

    with TileContext(nc) as tc:
        with tc.tile_pool(name="const", bufs=1) as cpool:
            R_t = cpool.tile([128, KH, HD], BF16)
            Rs_t = cpool.tile([128, T * C, KP, PD], BF16)
            ident = cpool.tile([128, 128], BF16)
            qT = [cpool.tile([128, 2, BZ], BF16, name=f"qT{v}") for v in range(T * U)]
            recq = cpool.tile([128, T * C, QC, S], F32)
            rm = [cpool.tile([128, T * U * QC], F32, name=f"rm{i}") for i in range(2)]
            O = cpool.tile([128, T * U, QC], F32)
            neg = cpool.tile([128, GK], BF16)
            nc.vector.memset(neg[:], -10.0)

            nc.sync.dma_start(out=R_t[:], in_=R[:].rearrange("(k p) m -> p k m", p=128))
            nc.sync.dma_start(out=Rs_t[:],
                              in_=Rs[:].rearrange("t c (k p) e -> p (t c) k e", p=128))
            make_identity(nc, ident[:])
            nc.vector.memset(rm[0][:], -2.0)

            # ---------------- query side (once) ----------------
            with tc.tile_pool(name="qstage", bufs=1) as qsb, \
                 tc.tile_pool(name="qpsum", bufs=2, space="PSUM") as qps:
                hT_t = qsb.tile([128, KH, BZ], BF16)
                hrT_t = qsb.tile([128, MC, BZ], BF16)
                nc.sync.dma_start(out=hT_t[:],
                                  in_=hT[:].rearrange("(k p) q -> p k q", p=128))
                for m in range(MC):
                    for g in range(2):
                        hr_ps = qps.tile([128, 512], F32, tag="hr_ps")
                        for k in range(KH):
                            nc.tensor.matmul(
                                hr_ps[:],
                                lhsT=R_t[:, k, m * 128:(m + 1) * 128],
                                rhs=hT_t[:, k, g * 512:(g + 1) * 512],
                                start=(k == 0), stop=(k == KH - 1))
                        nc.scalar.copy(out=hrT_t[:, m, g * 512:(g + 1) * 512],
                                       in_=hr_ps[:])
                for t in range(T):
                    for c in range(C):
                        for qc in range(QC):
                            zq_ps = qps.tile([128, PD], F32, tag="zq_ps")
                            for k in range(KP):
                                nc.tensor.matmul(
                                    zq_ps[:],
                                    lhsT=hrT_t[:, c * KP + k, qc * 128:(qc + 1) * 128],
                                    rhs=Rs_t[:, t * C + c, k, :],
                                    start=(k == 0), stop=(k == KP - 1))
                            qn2 = qsb.tile([128, S], F32, tag="qn2", bufs=3)
                            qsq = qsb.tile([128, SD], F32, tag="qsq", bufs=2)
                            for s in range(S):
                                nc.scalar.activation(
                                    out=qsq[:], in_=zq_ps[:, s * SD:(s + 1) * SD],
                                    func=AF.Square, accum_out=qn2[:, s:s + 1])
                            qsr = qsb.tile([128, S], F32, tag="qsr", bufs=3)
                            nc.scalar.sqrt(out=qsr[:], in_=qn2[:])
                            nc.vector.reciprocal(
                                out=recq[:, t * C + c, qc, :], in_=qsr[:])
                            zq_b = qsb.tile([128, PD], BF16, tag="zq_b", bufs=3)
                            nc.scalar.copy(out=zq_b[:], in_=zq_ps[:])
                            for s in range(S):
                                v = t * U + c * S + s
                                qt_ps = qps.tile([128, 2, 128], BF16, tag="qt_ps")
                                for sdc in range(2):
                                    off = s * SD + sdc * 128
                                    nc.tensor.transpose(
                                        qt_ps[:, sdc, :],
                                        zq_b[:, off:off + 128], ident[:])
                                nc.scalar.copy(
                                    out=qT[v][:, :, qc * 128:(qc + 1) * 128],
                                    in_=qt_ps[:])

            # ---------------- key-side streaming loop ----------------
            with tc.tile_pool(name="kstream", bufs=2) as ksb, \
                 tc.tile_pool(name="ksmall", bufs=3) as ksm, \
                 tc.tile_pool(name="knTp", bufs=1) as knp, \
                 tc.tile_pool(name="kpsum", bufs=2, space="PSUM") as kps:
                knT = [knp.tile([128, 2, GK], BF16, name=f"knT{v}")
                       for v in range(T * U)]
                for kg in range(n_kg):
                    kbT_t = ksb.tile([128, KH, GK], BF16, tag="kbT_t")
                    nc.sync.dma_start(
                        out=kbT_t[:],
                        in_=kbT[:].rearrange("(k p) l -> p k l", p=128)
                              [:, :, kg * GK:(kg + 1) * GK])
                    xrT_t = ksb.tile([128, MC, GK], BF16, tag="xrT_t")
                    for m in range(MC):
                        xr_ps = kps.tile([128, GK], F32, tag="xr_ps")
                        for k in range(KH):
                            nc.tensor.matmul(
                                xr_ps[:],
                                lhsT=R_t[:, k, m * 128:(m + 1) * 128],
                                rhs=kbT_t[:, k, :],
                                start=(k == 0), stop=(k == KH - 1))
                        nc.scalar.copy(out=xrT_t[:, m, :], in_=xr_ps[:])
                    for t in range(T):
                        for c in range(C):
                            for kc in range(KC):
                                z_ps = kps.tile([128, PD], F32, tag="z_ps")
                                for k in range(KP):
                                    nc.tensor.matmul(
                                        z_ps[:],
                                        lhsT=xrT_t[:, c * KP + k,
                                                   kc * 128:(kc + 1) * 128],
                                        rhs=Rs_t[:, t * C + c, k, :],
                                        start=(k == 0), stop=(k == KP - 1))
                                kn2 = ksm.tile([128, S], F32, tag="kn2")
                                ksq = ksm.tile([128, SD], F32, tag="ksq", bufs=2)
                                for s in range(S):
                                    nc.scalar.activation(
                                        out=ksq[:], in_=z_ps[:, s * SD:(s + 1) * SD],
                                        func=AF.Square, accum_out=kn2[:, s:s + 1])
                                ksr = ksm.tile([128, S], F32, tag="ksr")
                                nc.scalar.sqrt(out=ksr[:], in_=kn2[:])
                                krc = ksm.tile([128, S], F32, tag="krc")
                                nc.vector.reciprocal(out=krc[:], in_=ksr[:])
                                kn_b = ksm.tile([128, PD], BF16, tag="kn_b")
                                for s in range(S):
                                    nc.scalar.mul(
                                        out=kn_b[:, s * SD:(s + 1) * SD],
                                        in_=z_ps[:, s * SD:(s + 1) * SD],
                                        mul=krc[:, s:s + 1])
                                for s in range(S):
                                    v = t * U + c * S + s
                                    kt_ps = kps.tile([128, 2, 128], BF16,
                                                     tag="kt_ps")
                                    for sdc in range(2):
                                        off = s * SD + sdc * 128
                                        nc.tensor.transpose(
                                            kt_ps[:, sdc, :],
                                            kn_b[:, off:off + 128], ident[:])
                                    nc.scalar.copy(
                                        out=knT[v][:, :, kc * 128:(kc + 1) * 128],
                                        in_=kt_ps[:])
                    for v in range(T * U):
                        for qc in range(QC):
                            sim_ps = kps.tile([128, GK], F32, tag="sim_ps")
                            for sdc in range(2):
                                nc.tensor.matmul(
                                    sim_ps[:],
                                    lhsT=qT[v][:, sdc, qc * 128:(qc + 1) * 128],
                                    rhs=knT[v][:, sdc, :],
                                    start=(sdc == 0), stop=(sdc == 1))
                            col = v * QC + qc
                            if use_ttr:
                                ttr_scr = ksm.tile([128, GK], BF16,
                                                   tag="ttr_scr", bufs=2)
                                nc.vector.tensor_tensor_reduce(
                                    out=ttr_scr[:],
                                    in0=sim_ps[:], in1=neg[:],
                                    scale=1.0,
                                    scalar=rm[kg % 2][:, col:col + 1],
                                    op0=mybir.AluOpType.max,
                                    op1=mybir.AluOpType.max,
                                    accum_out=rm[(kg + 1) % 2][:, col:col + 1])
                            else:
                                mtmp = ksm.tile([128, 1], F32, tag="mtmp",
                                                bufs=4)
                                nc.vector.reduce_max(
                                    out=mtmp[:], in_=sim_ps[:],
                                    axis=mybir.AxisListType.X)
                                nc.vector.tensor_tensor(
                                    out=rm[(kg + 1) % 2][:, col:col + 1],
                                    in0=mtmp[:],
                                    in1=rm[kg % 2][:, col:col + 1],
                                    op=mybir.AluOpType.max)

            # -------- finalize: fold in 1/||q|| (positive, commutes w/ max) --
            for t in range(T):
                for c in range(C):
                    for s in range(S):
                        v = t * U + c * S + s
                        for qc in range(QC):
                            col = v * QC + qc
                            nc.vector.tensor_tensor(
                                out=O[:, v, qc:qc + 1],
                                in0=rm[n_kg % 2][:, col:col + 1],
                                in1=recq[:, t * C + c, qc, s:s + 1],
                                op=mybir.AluOpType.mult)
            nc.sync.dma_start(out=y[:], in_=O[:].rearrange("p v c -> p (v c)"))
    return nc


def make_in_maps(h, keys, previous_R, Rs):
    Rb = previous_R.astype(BF)
    Rsb = Rs.astype(BF)
    hTb = np.ascontiguousarray(h.T).astype(BF)
    in_maps = []
    for i in range(NCORES):
        in_maps.append({
            "kbT": np.ascontiguousarray(keys[i].T).astype(BF),
            "R": Rb,
            "Rs": Rsb,
            "hT": hTb,
        })
    return in_maps


def unpack_y(y):
    """[128, T*U*QC] device layout -> [T*U, BZ]."""
    return np.asarray(y, np.float32).reshape(128, T * U, QC).transpose(1, 2, 0) \
             .reshape(T * U, BZ)


def reduce_outputs(results):
    parts = np.stack([unpack_y(r["y"]) for r in results])
    allmax = parts.max(axis=0)                     # [T*U, BZ]
    loss = -(allmax.mean(axis=-1).sum() * SD / HD)
    return np.float32(loss)


def kernel(h, keys, previous_R, Rs):
    h = np.asarray(h, np.float32)
    keys = np.asarray(keys, np.float32)
    previous_R = np.asarray(previous_R, np.float32)
    Rs = np.asarray(Rs, np.float32)
    in_maps = make_in_maps(h, keys, previous_R, Rs)
    nc = build_program()
    nc.finalize()
    res = run_bass_kernel_spmd(nc, in_maps, list(range(NCORES)))
    return reduce_outputs(res.results)
